# revision 12
# baseline (speedup 1.0000x reference)
"""Trainium2 Bass kernel for nn_ByteSequenceEmbedder.

Model (per sequence, 8 sequences data-parallel over 8 NeuronCores):
  x  = tok_emb[tokens] + bpe*E[4] + word*E[3]                 [T=4096, 64]
  x  = relu(conv3(x, W0) + b0); 2x highway(512)               [T, 512]
  x  = relu(conv3(x, W1) + b1 + x); 2x highway(512)           [T, 512]
  x  = per-word segment max (ragged, sorted seg_ids, W=1024)  [W, 512]
  out= x @ Pw + Pb                                            [W, 512]

Device strategy (per core, feature-major layout [feat partitions x tokens]):
 - embedding via one dma_gather(transpose=True) from a combined 1056-row
   bf16 table indexed by tok + 264*(bpe + 2*word)  (markers folded in)
 - conv/highway/proj as bf16 matmuls (fp32 PSUM accumulation), biases applied
   on the PSUM->SBUF eviction (per-partition bias slot); conv SAME-padding via
   clipped edge matmuls (center tap starts each accumulation group)
 - conv1 residual folded into the center conv tap (W1[1] += I)
 - highway combine y' = y + g*(h-y) on DVE (3 tensor_tensor ops)
 - segment max: PE-transpose y1 to token-major, bounce via DRAM, per
   128-word chunk a transpose-mode dma_gather of ntaps clamped row indices
   (idx = min(start_w+j, end_w); duplicate rows don't change the max) which
   lands feature-major; DVE max tree writes the proj lhsT directly
 - proj + bias (ones-row matmul), fp32 out

HW quirks honored: transpose-mode dma_gather must write an exact-fit,
offset-free SBUF tile and needs single_packet=False beyond ~512 indices.
"""

import functools
import os
import sys

import numpy as np

for _p in ("/opt/trn_rl_repo", "/root/.axon_site/_ro/trn_rl_repo"):
    if os.path.isdir(_p) and _p not in sys.path:
        sys.path.append(_p)

import ml_dtypes  # noqa: E402

from contextlib import ExitStack  # noqa: E402

from concourse import bacc, bass, mybir, tile  # noqa: E402
from concourse import library_config  # noqa: E402
from concourse.bass_utils import run_bass_kernel_spmd  # noqa: E402

B, T, W = 8, 4096, 1024
DB, DW = 64, 512
NH = 2
VOCAB = 264
BPE_MARK, WORD_MARK = 4, 3
SC = 2048          # tokens per super-chunk (psum tile free size)
NSC = T // SC
NBANK = 512        # fp32 elems per PSUM bank (one matmul N)
MCH = DW // 128    # output-feature chunks
KCH = DW // 128    # contraction chunks
NCORES = 8
CVOCAB = 4 * VOCAB  # combined (tok, bpe, word) vocabulary

BF16 = mybir.dt.bfloat16
F32 = mybir.dt.float32
I16 = mybir.dt.int16
AF = mybir.ActivationFunctionType
OP = mybir.AluOpType

bf16_np = ml_dtypes.bfloat16


def _col_bg(block, l):
    return 8 + block * 8 + l * 4


def _col_bh(block, l):
    return 24 + block * 8 + l * 4


def build_program(ntaps: int, stage: int = 8) -> bass.Bass:
    nc = bacc.Bacc("TRN2", target_bir_lowering=False, debug=False)

    def din(name, shape, dtype):
        return nc.dram_tensor(name, list(shape), dtype, kind="ExternalInput")

    emb_d = din("emb_comb", (CVOCAB, 128), BF16)
    tokidx_d = din("tok_idx", (128, T // 16), I16)
    w0_d = din("w0", (DB, 3, DW), BF16)
    w1_d = din("w1", (128, 3, KCH, DW), BF16)
    wg0_d = din("wg0", (128, NH, KCH, DW), BF16)
    wh0_d = din("wh0", (128, NH, KCH, DW), BF16)
    wg1_d = din("wg1", (128, NH, KCH, DW), BF16)
    wh1_d = din("wh1", (128, NH, KCH, DW), BF16)
    projw_d = din("projw", (128, KCH, DW), BF16)
    projb_d = din("projb", (1, DW), BF16)
    bias_d = din("biases", (128, 40), F32)
    ident_d = din("ident", (128, 128), BF16)
    gidx_d = din("gidx", (128, 8 * ntaps * 8), I16)
    out_d = nc.dram_tensor("out", [W, DW], F32, kind="ExternalOutput")
    if stage < 8:
        dbg_d = nc.dram_tensor("dbg", [5, 128, T], BF16, kind="ExternalOutput")
    y1t_d = nc.dram_tensor(
        "y1t", [T, DW], BF16,
        kind="Internal" if stage >= 8 else "ExternalOutput")  # scratch

    with tile.TileContext(nc) as tc, ExitStack() as ctx:
        const = ctx.enter_context(tc.tile_pool(name="const", bufs=1))
        ps = ctx.enter_context(tc.tile_pool(name="psp", bufs=2, space="PSUM"))
        gp = ctx.enter_context(tc.tile_pool(name="gpool", bufs=4))
        hp = ctx.enter_context(tc.tile_pool(name="hpool", bufs=4))
        dp = ctx.enter_context(tc.tile_pool(name="dpool", bufs=2))
        y1p = ctx.enter_context(tc.tile_pool(name="y1pool", bufs=8))
        tp = ctx.enter_context(tc.tile_pool(name="tpool", bufs=3))
        gat = ctx.enter_context(tc.tile_pool(name="gat", bufs=2))
        obp = ctx.enter_context(tc.tile_pool(name="obp", bufs=2))

        nc.gpsimd.load_library(library_config.mlp)

        def load(dram_t, shape, dtype, name):
            t = const.tile(shape, dtype, name=name)
            nc.sync.dma_start(out=t[:], in_=dram_t[:])
            return t

        w0_sb = load(w0_d, [DB, 3, DW], BF16, "w0_sb")
        w1_sb = load(w1_d, [128, 3, KCH, DW], BF16, "w1_sb")
        wg0_sb = load(wg0_d, [128, NH, KCH, DW], BF16, "wg0_sb")
        wh0_sb = load(wh0_d, [128, NH, KCH, DW], BF16, "wh0_sb")
        wg1_sb = load(wg1_d, [128, NH, KCH, DW], BF16, "wg1_sb")
        wh1_sb = load(wh1_d, [128, NH, KCH, DW], BF16, "wh1_sb")
        projw_sb = load(projw_d, [128, KCH, DW], BF16, "projw_sb")
        projb_sb = load(projb_d, [1, DW], BF16, "projb_sb")
        bias_sb = load(bias_d, [128, 40], F32, "bias_sb")
        tokidx_sb = load(tokidx_d, [128, T // 16], I16, "tokidx_sb")
        gidx_sb = load(gidx_d, [128, 8 * ntaps * 8], I16, "gidx_sb")
        ident_sb = load(ident_d, [128, 128], BF16, "ident_sb")
        ones_sb = const.tile([1, 128], BF16, name="ones_sb")
        nc.vector.memset(ones_sb[:], 1.0)

        # ---- embedding gather: xg[p, t] = emb_comb[cidx[t], p] ----
        xg = const.tile([128, T], BF16, name="xg")
        if stage >= 1:
            nc.gpsimd.dma_gather(
                out_ap=xg[:].rearrange("p (c n) -> p c n", c=1),
                in_ap=emb_d[:],
                idxs_ap=tokidx_sb[:],
                num_idxs=T,
                num_idxs_reg=T,
                elem_size=128,
                transpose=True,
                single_packet=False,
            )
        else:
            nc.vector.memset(xg[:], 0.0)

        y0_sb = [const.tile([128, T], BF16, name=f"y0_{m}") for m in range(MCH)]

        def conv_taps(pc, base, lhsT_of, rhs_of, nk):
            """Accumulate a 3-tap SAME conv into psum tile pc [128, SC].

            lhsT_of(k, kc) -> weight AP; rhs_of(kc, lo, ln) -> input AP over
            tokens [lo, lo+ln). Center tap issued first so every psum column
            is initialized by a start=True matmul; edge taps are clipped."""
            order = [(1, kc) for kc in range(nk)] + \
                    [(0, kc) for kc in range(nk)] + \
                    [(2, kc) for kc in range(nk)]
            last = order[-1]
            for (k, kc) in order:
                for n in range(SC // NBANK):
                    t0 = base + n * NBANK
                    col0, col1 = n * NBANK, (n + 1) * NBANK
                    lo = t0 + (k - 1)
                    ln = NBANK
                    o0, o1 = col0, col1
                    if lo < 0:
                        lo, ln, o0 = 0, NBANK - 1, col0 + 1
                    elif lo + ln > T:
                        ln, o1 = T - lo, col1 - 1
                    nc.tensor.matmul(
                        out=pc[:, o0:o1],
                        lhsT=lhsT_of(k, kc),
                        rhs=rhs_of(kc, lo, ln),
                        start=(k == 1 and kc == 0),
                        stop=((k, kc) == last),
                    )

        # ---- conv0, relu ----
        for sc in range(NSC if stage >= 2 else 0):
            base = sc * SC
            for m in range(MCH):
                pc = ps.tile([128, SC], F32, tag="ps", name="pc")
                conv_taps(
                    pc, base,
                    lambda k, kc: w0_sb[:, k, m * 128:(m + 1) * 128],
                    lambda kc, lo, ln: xg[0:DB, lo:lo + ln],
                    nk=1,
                )
                nc.scalar.activation(
                    out=y0_sb[m][:, base:base + SC], in_=pc[:],
                    func=AF.Relu, bias=bias_sb[:, m:m + 1],
                )

        # ---- highway helper ----
        def hw_phase(wg_sb, wh_sb, l, colg, colh, y_ap):
            for sc in range(NSC):
                g_tiles = []
                for m in range(MCH):
                    pg = ps.tile([128, SC], F32, tag="ps", name="pg")
                    for k in range(KCH):
                        for n in range(SC // NBANK):
                            nc.tensor.matmul(
                                out=pg[:, n * NBANK:(n + 1) * NBANK],
                                lhsT=wg_sb[:, l, k, m * 128:(m + 1) * 128],
                                rhs=y_ap(k, sc)[:, n * NBANK:(n + 1) * NBANK],
                                start=(k == 0),
                                stop=(k == KCH - 1),
                            )
                    g = gp.tile([128, SC], BF16, tag="g", name="g")
                    nc.scalar.activation(
                        out=g[:], in_=pg[:], func=AF.Sigmoid,
                        bias=bias_sb[:, colg + m:colg + m + 1],
                    )
                    g_tiles.append(g)
                h_tiles = []
                for m in range(MCH):
                    ph = ps.tile([128, SC], F32, tag="ps", name="ph")
                    for k in range(KCH):
                        for n in range(SC // NBANK):
                            nc.tensor.matmul(
                                out=ph[:, n * NBANK:(n + 1) * NBANK],
                                lhsT=wh_sb[:, l, k, m * 128:(m + 1) * 128],
                                rhs=y_ap(k, sc)[:, n * NBANK:(n + 1) * NBANK],
                                start=(k == 0),
                                stop=(k == KCH - 1),
                            )
                    h = hp.tile([128, SC], BF16, tag="h", name="h")
                    nc.scalar.activation(
                        out=h[:], in_=ph[:], func=AF.Relu,
                        bias=bias_sb[:, colh + m:colh + m + 1],
                    )
                    h_tiles.append(h)
                for m in range(MCH):
                    yap = y_ap(m, sc)
                    d = dp.tile([128, SC], BF16, tag="d", name="d")
                    nc.vector.tensor_tensor(
                        out=d[:], in0=h_tiles[m][:], in1=yap, op=OP.subtract)
                    nc.vector.tensor_tensor(
                        out=d[:], in0=g_tiles[m][:], in1=d[:], op=OP.mult)
                    nc.vector.tensor_tensor(
                        out=yap, in0=yap, in1=d[:], op=OP.add)

        def y0_ap(m, sc):
            return y0_sb[m][:, sc * SC:sc * SC + SC]

        for l in range(NH if stage >= 3 else 0):
            hw_phase(wg0_sb, wh0_sb, l, _col_bg(0, l), _col_bh(0, l), y0_ap)

        # ---- conv1 (residual folded into center tap), relu ----
        y1_tiles = {}
        for sc in range(NSC if stage >= 4 else 0):
            base = sc * SC
            for m in range(MCH):
                pc = ps.tile([128, SC], F32, tag="ps", name="pc1")
                conv_taps(
                    pc, base,
                    lambda k, kc: w1_sb[:, k, kc, m * 128:(m + 1) * 128],
                    lambda kc, lo, ln: y0_sb[kc][:, lo:lo + ln],
                    nk=KCH,
                )
                y1 = y1p.tile([128, SC], BF16, tag="y1", name=f"y1_{m}_{sc}")
                nc.scalar.activation(
                    out=y1[:], in_=pc[:], func=AF.Relu,
                    bias=bias_sb[:, 4 + m:5 + m],
                )
                y1_tiles[(m, sc)] = y1

        def y1_ap(m, sc):
            return y1_tiles[(m, sc)][:]

        for l in range(NH if stage >= 5 else 0):
            hw_phase(wg1_sb, wh1_sb, l, _col_bg(1, l), _col_bh(1, l), y1_ap)

        # ---- transpose y1 -> token-major, bounce to DRAM ----
        for sc in range(NSC if stage >= 6 else 0):
            for i in range(SC // 128):
                pt = ps.tile([128, 512], BF16, tag="ps", name="pt")
                for m in range(MCH):
                    nc.tensor.transpose(
                        out=pt[:, m * 128:(m + 1) * 128],
                        in_=y1_tiles[(m, sc)][:, i * 128:(i + 1) * 128],
                        identity=ident_sb[:],
                    )
                st = tp.tile([128, 512], BF16, tag="y1t", name="st")
                nc.vector.tensor_copy(out=st[:], in_=pt[:])
                row0 = (sc * (SC // 128) + i) * 128
                nc.sync.dma_start(out=y1t_d[row0:row0 + 128, :], in_=st[:])

        # ---- per word-chunk: transpose-mode gather of ntaps rows + max tree
        # out[p, c, i] = y1t[idx[i]][c*128+p]; idx[j*128+wl] = clamp(s+j, e)
        a2_all = const.tile([128, KCH, W], BF16, name="a2_all")
        for wc in range(8 if stage >= 7 else 0):
            tap = gat.tile([128, KCH, ntaps * 128], BF16, tag="tap", name="tap")
            nc.gpsimd.dma_gather(
                out_ap=tap[:],
                in_ap=y1t_d[:],
                idxs_ap=gidx_sb[:, wc * ntaps * 8:(wc + 1) * ntaps * 8],
                num_idxs=ntaps * 128,
                num_idxs_reg=ntaps * 128,
                elem_size=DW,
                transpose=True,
                single_packet=False,
            )
            a2s = a2_all[:, :, wc * 128:(wc + 1) * 128]
            nc.vector.tensor_tensor(
                out=a2s, in0=tap[:, :, 0:128], in1=tap[:, :, 128:256], op=OP.max)
            for j in range(2, ntaps):
                nc.vector.tensor_tensor(
                    out=a2s, in0=a2s, in1=tap[:, :, j * 128:(j + 1) * 128],
                    op=OP.max)

        if stage < 8:
            nc.sync.dma_start(out=dbg_d[4], in_=xg[:])
            for m in range(MCH):
                nc.sync.dma_start(out=dbg_d[m], in_=y0_sb[m][:])

        # ---- proj + bias (ones-row matmul), fp32 out ----
        for wc in range(8 if stage >= 8 else 0):
            po = ps.tile([128, DW], F32, tag="ps", name="po")
            for k in range(KCH):
                nc.tensor.matmul(
                    out=po[:],
                    lhsT=a2_all[:, k, wc * 128:(wc + 1) * 128],
                    rhs=projw_sb[:, k, :],
                    start=(k == 0),
                    stop=False,
                )
            nc.tensor.matmul(
                out=po[:], lhsT=ones_sb[:, 0:128], rhs=projb_sb[:],
                start=False, stop=True,
            )
            ob = obp.tile([128, DW], F32, tag="ob", name="ob")
            nc.vector.tensor_copy(out=ob[:], in_=po[:])
            nc.sync.dma_start(out=out_d[wc * 128:(wc + 1) * 128, :], in_=ob[:])

    nc.compile()
    return nc


@functools.lru_cache(maxsize=2)
def _program(ntaps: int) -> bass.Bass:
    return build_program(ntaps)


def _pack_idx(lin: np.ndarray) -> np.ndarray:
    """SWDGE idx layout: [128, N/16] int16, value n at [p, s] with
    n = s*16 + p%16, replicated across the eight 16-partition groups."""
    n = len(lin)
    assert n % 16 == 0
    arr = np.asarray(lin, dtype=np.int16).reshape(n // 16, 16).T  # [16, n/16]
    return np.tile(arr, (8, 1)).copy()


def prepare(inputs):
    f32 = np.float32
    bt = np.asarray(inputs["byte_tokens"]).astype(np.int64)
    bpe = np.asarray(inputs["bpe_mask"]).astype(np.int64)
    wrd = np.asarray(inputs["word_mask"]).astype(np.int64)
    seg = np.asarray(inputs["seg_ids"]).astype(np.int64)
    emb = np.asarray(inputs["tok_emb"], dtype=f32)
    conv0_w = np.asarray(inputs["conv0_w"], dtype=f32)
    conv0_b = np.asarray(inputs["conv0_b"], dtype=f32)
    conv1_w = np.asarray(inputs["conv1_w"], dtype=f32)
    conv1_b = np.asarray(inputs["conv1_b"], dtype=f32)
    hw_w = {
        (0, "g"): np.asarray(inputs["hw0_wg"], dtype=f32),
        (0, "h"): np.asarray(inputs["hw0_wh"], dtype=f32),
        (1, "g"): np.asarray(inputs["hw1_wg"], dtype=f32),
        (1, "h"): np.asarray(inputs["hw1_wh"], dtype=f32),
    }
    hw_b = {
        (0, "g"): np.asarray(inputs["hw0_bg"], dtype=f32),
        (0, "h"): np.asarray(inputs["hw0_bh"], dtype=f32),
        (1, "g"): np.asarray(inputs["hw1_bg"], dtype=f32),
        (1, "h"): np.asarray(inputs["hw1_bh"], dtype=f32),
    }
    proj_w = np.asarray(inputs["proj_w"], dtype=f32)
    proj_b = np.asarray(inputs["proj_b"], dtype=f32)

    def as_bf16(x):
        return np.ascontiguousarray(x.astype(bf16_np))

    # combined embedding table: row v + 264*(b + 2*w) = E[v] + b*E4 + w*E3
    embc = np.zeros((CVOCAB, 128), f32)
    for bm in range(2):
        for wm in range(2):
            r0 = VOCAB * (bm + 2 * wm)
            embc[r0:r0 + VOCAB, :DB] = (
                emb + bm * emb[BPE_MARK] + wm * emb[WORD_MARK])
    c1 = conv1_w.copy()
    c1[1] += np.eye(DW, dtype=f32)

    def chunk_kl(wm):  # [L, 512, 512] -> [128, L, 4, 512]
        L = wm.shape[0]
        return np.transpose(wm.reshape(L, KCH, 128, DW), (2, 0, 1, 3))

    shared = {
        "emb_comb": as_bf16(embc),
        "w0": as_bf16(np.transpose(conv0_w, (1, 0, 2))),          # [64,3,512]
        "w1": as_bf16(np.transpose(c1.reshape(3, KCH, 128, DW), (2, 0, 1, 3))),
        "wg0": as_bf16(chunk_kl(hw_w[(0, "g")])),
        "wh0": as_bf16(chunk_kl(hw_w[(0, "h")])),
        "wg1": as_bf16(chunk_kl(hw_w[(1, "g")])),
        "wh1": as_bf16(chunk_kl(hw_w[(1, "h")])),
        "projw": as_bf16(np.transpose(proj_w.reshape(KCH, 128, DW), (1, 0, 2))),
        "projb": as_bf16(proj_b.reshape(1, DW)),
        "ident": np.eye(128, dtype=bf16_np),
    }

    bias_h = np.zeros((128, 40), f32)
    bias_h[:, 0:4] = conv0_b.reshape(KCH, 128).T
    bias_h[:, 4:8] = conv1_b.reshape(KCH, 128).T
    for blk in (0, 1):
        for l in range(NH):
            bias_h[:, _col_bg(blk, l):_col_bg(blk, l) + 4] = \
                hw_b[(blk, "g")][l].reshape(KCH, 128).T
            bias_h[:, _col_bh(blk, l):_col_bh(blk, l) + 4] = \
                hw_b[(blk, "h")][l].reshape(KCH, 128).T
    shared["biases"] = np.ascontiguousarray(bias_h)

    # per-core seg prep; ntaps = max segment length over the whole batch
    counts = np.zeros((B, W), np.int64)
    for b in range(B):
        counts[b] = np.bincount(seg[b], minlength=W)[:W]
    assert (counts >= 1).all(), "empty segments unsupported"
    ntaps = max(int(counts.max()), 2)
    starts = np.zeros((B, W), np.int64)
    starts[:, 1:] = np.cumsum(counts, axis=1)[:, :-1]
    ends = starts + counts - 1

    in_maps = []
    for b in range(B):
        cidx = bt[b] + VOCAB * (bpe[b] + 2 * wrd[b])
        gl = np.empty(8 * ntaps * 128, np.int64)
        for wc in range(8):
            nvec = np.arange(ntaps * 128)
            wv = wc * 128 + (nvec % 128)
            jv = nvec // 128
            gl[wc * ntaps * 128:(wc + 1) * ntaps * 128] = np.minimum(
                starts[b, wv] + jv, ends[b, wv]
            )
        m = dict(shared)
        m["tok_idx"] = _pack_idx(cidx)
        m["gidx"] = np.concatenate(
            [_pack_idx(gl[wc * ntaps * 128:(wc + 1) * ntaps * 128])
             for wc in range(8)], axis=1
        ).copy()
        in_maps.append(m)
    return ntaps, in_maps


def _run(inputs, trace=False, **kwargs):
    ntaps, in_maps = prepare(inputs)
    nc = _program(ntaps)
    res = run_bass_kernel_spmd(
        nc, in_maps, core_ids=list(range(NCORES)), trace=trace, **kwargs
    )
    out = np.stack([res.results[b]["out"] for b in range(B)], axis=0)
    return out.astype(np.float32), res


def kernel(**inputs) -> np.ndarray:
    out, _ = _run(inputs, trace=False)
    return out


def run_traced(inputs, **kwargs):
    return _run(inputs, trace=True, **kwargs)


# revision 22
# speedup vs baseline: 234.8225x; 234.8225x over previous
"""Trainium2 Bass kernel for nn_ByteSequenceEmbedder.

Model (per sequence, 8 sequences data-parallel over 8 NeuronCores):
  x  = tok_emb[tokens] + bpe*E[4] + word*E[3]                 [T=4096, 64]
  x  = relu(conv3(x, W0) + b0); 2x highway(512)               [T, 512]
  x  = relu(conv3(x, W1) + b1 + x); 2x highway(512)           [T, 512]
  x  = per-word segment max (ragged, sorted seg_ids, W=1024)  [W, 512]
  out= x @ Pw + Pb                                            [W, 512]

Device strategy (per core, feature-major layout [feat partitions x tokens]):
 - embedding via one dma_gather(transpose=True) from a combined 1056-row
   bf16 table indexed by tok + 264*(bpe + 2*word)  (markers folded in)
 - conv/highway/proj as bf16 matmuls (fp32 PSUM accumulation), biases applied
   on the PSUM->SBUF eviction (per-partition bias slot); conv SAME-padding via
   clipped edge matmuls (center tap starts each accumulation group)
 - conv1 residual folded into the center conv tap (W1[1] += I)
 - highway combine y' = y + g*(h-y) on DVE (3 tensor_tensor ops)
 - segment max: PE-transpose y1 to token-major, bounce via DRAM, per
   128-word chunk a transpose-mode dma_gather of ntaps clamped row indices
   (idx = min(start_w+j, end_w); duplicate rows don't change the max) which
   lands feature-major; DVE max tree writes the proj lhsT directly
 - proj + bias (ones-row matmul), fp32 out

HW quirks honored: transpose-mode dma_gather must write an exact-fit,
offset-free SBUF tile and needs single_packet=False beyond ~512 indices.
"""

import functools
import os
import sys

import numpy as np

for _p in ("/opt/trn_rl_repo", "/root/.axon_site/_ro/trn_rl_repo"):
    if os.path.isdir(_p) and _p not in sys.path:
        sys.path.append(_p)

import ml_dtypes  # noqa: E402

from contextlib import ExitStack  # noqa: E402

from concourse import bacc, bass, mybir, tile  # noqa: E402
from concourse import library_config  # noqa: E402
from concourse.bass_utils import run_bass_kernel_spmd  # noqa: E402

B, T, W = 8, 4096, 1024
DB, DW = 64, 512
NH = 2
VOCAB = 264
BPE_MARK, WORD_MARK = 4, 3
SC = 2048          # tokens per super-chunk (psum tile free size)
NSC = T // SC
NBANK = 512        # fp32 elems per PSUM bank
NMM = 512          # matmul moving-operand columns (PSUM bank limit)
MCH = DW // 128    # output-feature chunks
KCH = DW // 128    # contraction chunks
NCORES = 8
CVOCAB = 4 * VOCAB  # combined (tok, bpe, word) vocabulary

BF16 = mybir.dt.bfloat16
F32 = mybir.dt.float32
I16 = mybir.dt.int16
AF = mybir.ActivationFunctionType
OP = mybir.AluOpType

bf16_np = ml_dtypes.bfloat16


def _col_bg(block, l):
    return 8 + block * 8 + l * 4


def _col_bh(block, l):
    return 24 + block * 8 + l * 4


def build_program(ntaps: int, stage: int = 8) -> bass.Bass:
    nc = bacc.Bacc("TRN2", target_bir_lowering=False, debug=False)

    def din(name, shape, dtype):
        return nc.dram_tensor(name, list(shape), dtype, kind="ExternalInput")

    emb_d = din("emb_comb", (CVOCAB, 128), BF16)
    tokidx_d = din("tok_idx", (128, T // 16), I16)
    w0_d = din("w0", (DB, 3, DW), BF16)
    w1_d = din("w1", (128, 3, KCH, DW), BF16)
    wg0_d = din("wg0", (128, NH, KCH, DW), BF16)
    wh0_d = din("wh0", (128, NH, KCH, DW), BF16)
    wg1_d = din("wg1", (128, NH, KCH, DW), BF16)
    wh1_d = din("wh1", (128, NH, KCH, DW), BF16)
    projw_d = din("projw", (128, KCH, DW), BF16)
    projb_d = din("projb", (1, DW), BF16)
    bias_d = din("biases", (128, 40), F32)
    ident_d = din("ident", (128, 128), BF16)
    gidx_d = din("gidx", (128, 8 * ntaps * 8), I16)
    out_d = nc.dram_tensor("out", [W, DW], F32, kind="ExternalOutput")
    if stage < 8:
        dbg_d = nc.dram_tensor("dbg", [5, 128, T], BF16, kind="ExternalOutput")
    y1t_d = nc.dram_tensor(
        "y1t", [T, DW], BF16,
        kind="Internal" if stage >= 8 else "ExternalOutput")  # scratch

    with tile.TileContext(nc) as tc, ExitStack() as ctx:
        const = ctx.enter_context(tc.tile_pool(name="const", bufs=1))
        ps = ctx.enter_context(tc.tile_pool(name="psp", bufs=2, space="PSUM"))
        gp = ctx.enter_context(tc.tile_pool(name="gpool", bufs=4))
        hp = ctx.enter_context(tc.tile_pool(name="hpool", bufs=4))
        dp = ctx.enter_context(tc.tile_pool(name="dpool", bufs=2))
        y1p = ctx.enter_context(tc.tile_pool(name="y1pool", bufs=8))
        tp = ctx.enter_context(tc.tile_pool(name="tpool", bufs=3))
        gat = ctx.enter_context(tc.tile_pool(name="gat", bufs=2))
        obp = ctx.enter_context(tc.tile_pool(name="obp", bufs=2))

        nc.gpsimd.load_library(library_config.mlp)

        def load(dram_t, shape, dtype, name):
            t = const.tile(shape, dtype, name=name)
            nc.sync.dma_start(out=t[:], in_=dram_t[:])
            return t

        # conv0 dependencies first (HWDGE is FIFO per engine): idx, w0, bias
        tokidx_sb = load(tokidx_d, [128, T // 16], I16, "tokidx_sb")
        w0_sb = load(w0_d, [DB, 3, DW], BF16, "w0_sb")
        bias_sb = load(bias_d, [128, 40], F32, "bias_sb")

        # ---- embedding gather: xg[p, t] = emb_comb[cidx[t], p] ----
        xg = const.tile([128, T], BF16, name="xg")
        if stage >= 1:
            # 4 chunked gathers (2KB-aligned output offsets) so conv0 can
            # start on chunk 0 while later chunks are still in flight
            EC = T // 4
            for r in range(4):
                nc.gpsimd.dma_gather(
                    out_ap=xg[:, r * EC:(r + 1) * EC].rearrange(
                        "p (c n) -> p c n", c=1),
                    in_ap=emb_d[:],
                    idxs_ap=tokidx_sb[:, r * (EC // 16):(r + 1) * (EC // 16)],
                    num_idxs=EC,
                    num_idxs_reg=EC,
                    elem_size=128,
                    transpose=True,
                    single_packet=False,
                )
        else:
            nc.vector.memset(xg[:], 0.0)

        wg0_sb = load(wg0_d, [128, NH, KCH, DW], BF16, "wg0_sb")
        wh0_sb = load(wh0_d, [128, NH, KCH, DW], BF16, "wh0_sb")
        w1_sb = load(w1_d, [128, 3, KCH, DW], BF16, "w1_sb")
        wg1_sb = load(wg1_d, [128, NH, KCH, DW], BF16, "wg1_sb")
        wh1_sb = load(wh1_d, [128, NH, KCH, DW], BF16, "wh1_sb")
        projw_sb = load(projw_d, [128, KCH, DW], BF16, "projw_sb")
        projb_sb = load(projb_d, [1, DW], BF16, "projb_sb")
        gidx_sb = load(gidx_d, [128, 8 * ntaps * 8], I16, "gidx_sb")
        ident_sb = load(ident_d, [128, 128], BF16, "ident_sb")
        ones_sb = const.tile([1, 128], BF16, name="ones_sb")
        nc.vector.memset(ones_sb[:], 1.0)

        y0_sb = [const.tile([128, T], BF16, name=f"y0_{m}") for m in range(MCH)]

        def conv_taps(pc, base, lhsT_of, rhs_of, nk):
            """Accumulate a 3-tap SAME conv into psum tile pc [128, SC].

            lhsT_of(k, kc) -> weight AP; rhs_of(kc, lo, ln) -> input AP over
            tokens [lo, lo+ln). Center tap issued first so every psum column
            is initialized by a start=True matmul; edge taps are clipped."""
            order = [(1, kc) for kc in range(nk)] + \
                    [(0, kc) for kc in range(nk)] + \
                    [(2, kc) for kc in range(nk)]
            last = order[-1]
            for (k, kc) in order:
                for n in range(SC // NMM):
                    t0 = base + n * NMM
                    col0, col1 = n * NMM, (n + 1) * NMM
                    lo = t0 + (k - 1)
                    ln = NMM
                    o0, o1 = col0, col1
                    if lo < 0:
                        lo, ln, o0 = 0, NMM - 1, col0 + 1
                    elif lo + ln > T:
                        ln, o1 = T - lo, col1 - 1
                    nc.tensor.matmul(
                        out=pc[:, o0:o1],
                        lhsT=lhsT_of(k, kc),
                        rhs=rhs_of(kc, lo, ln),
                        start=(k == 1 and kc == 0),
                        stop=((k, kc) == last),
                    )

        # ---- conv0, relu ----
        for sc in range(NSC if stage >= 2 else 0):
            base = sc * SC
            for m in range(MCH):
                pc = ps.tile([128, SC], F32, tag="ps", name="pc")
                conv_taps(
                    pc, base,
                    lambda k, kc: w0_sb[:, k, m * 128:(m + 1) * 128],
                    lambda kc, lo, ln: xg[0:DB, lo:lo + ln],
                    nk=1,
                )
                nc.scalar.activation(
                    out=y0_sb[m][:, base:base + SC], in_=pc[:],
                    func=AF.Relu, bias=bias_sb[:, m:m + 1],
                )

        # ---- highway helper ----
        def hw_phase(wg_sb, wh_sb, l, colg, colh, y_ap):
            for sc in range(NSC):
                g_tiles = []
                for m in range(MCH):
                    pg = ps.tile([128, SC], F32, tag="ps", name="pg")
                    for k in range(KCH):
                        for n in range(SC // NMM):
                            nc.tensor.matmul(
                                out=pg[:, n * NMM:(n + 1) * NMM],
                                lhsT=wg_sb[:, l, k, m * 128:(m + 1) * 128],
                                rhs=y_ap(k, sc)[:, n * NMM:(n + 1) * NMM],
                                start=(k == 0),
                                stop=(k == KCH - 1),
                            )
                    g = gp.tile([128, SC], BF16, tag="g", name="g")
                    nc.scalar.activation(
                        out=g[:], in_=pg[:], func=AF.Sigmoid,
                        bias=bias_sb[:, colg + m:colg + m + 1],
                    )
                    g_tiles.append(g)
                h_tiles = []
                for m in range(MCH):
                    ph = ps.tile([128, SC], F32, tag="ps", name="ph")
                    for k in range(KCH):
                        for n in range(SC // NMM):
                            nc.tensor.matmul(
                                out=ph[:, n * NMM:(n + 1) * NMM],
                                lhsT=wh_sb[:, l, k, m * 128:(m + 1) * 128],
                                rhs=y_ap(k, sc)[:, n * NMM:(n + 1) * NMM],
                                start=(k == 0),
                                stop=(k == KCH - 1),
                            )
                    h = hp.tile([128, SC], BF16, tag="h", name="h")
                    nc.scalar.activation(
                        out=h[:], in_=ph[:], func=AF.Relu,
                        bias=bias_sb[:, colh + m:colh + m + 1],
                    )
                    h_tiles.append(h)
                for m in range(MCH):
                    yap = y_ap(m, sc)
                    d = dp.tile([128, SC], BF16, tag="d", name="d")
                    nc.vector.tensor_tensor(
                        out=d[:], in0=h_tiles[m][:], in1=yap, op=OP.subtract)
                    nc.vector.tensor_tensor(
                        out=d[:], in0=g_tiles[m][:], in1=d[:], op=OP.mult)
                    nc.vector.tensor_tensor(
                        out=yap, in0=yap, in1=d[:], op=OP.add)

        def y0_ap(m, sc):
            return y0_sb[m][:, sc * SC:sc * SC + SC]

        for l in range(NH if stage >= 3 else 0):
            hw_phase(wg0_sb, wh0_sb, l, _col_bg(0, l), _col_bh(0, l), y0_ap)

        # ---- conv1 (residual folded into center tap), relu ----
        y1_tiles = {}
        for sc in range(NSC if stage >= 4 else 0):
            base = sc * SC
            for m in range(MCH):
                pc = ps.tile([128, SC], F32, tag="ps", name="pc1")
                conv_taps(
                    pc, base,
                    lambda k, kc: w1_sb[:, k, kc, m * 128:(m + 1) * 128],
                    lambda kc, lo, ln: y0_sb[kc][:, lo:lo + ln],
                    nk=KCH,
                )
                y1 = y1p.tile([128, SC], BF16, tag="y1", name=f"y1_{m}_{sc}")
                nc.scalar.activation(
                    out=y1[:], in_=pc[:], func=AF.Relu,
                    bias=bias_sb[:, 4 + m:5 + m],
                )
                y1_tiles[(m, sc)] = y1

        def y1_ap(m, sc):
            return y1_tiles[(m, sc)][:]

        for l in range(NH if stage >= 5 else 0):
            hw_phase(wg1_sb, wh1_sb, l, _col_bg(1, l), _col_bh(1, l), y1_ap)

        # ---- transpose y1 -> token-major, bounce to DRAM ----
        for sc in range(NSC if stage >= 6 else 0):
            for i in range(SC // 128):
                pt = ps.tile([128, 512], BF16, tag="ps", name="pt")
                for m in range(MCH):
                    nc.tensor.transpose(
                        out=pt[:, m * 128:(m + 1) * 128],
                        in_=y1_tiles[(m, sc)][:, i * 128:(i + 1) * 128],
                        identity=ident_sb[:],
                    )
                st = tp.tile([128, 512], BF16, tag="y1t", name="st")
                nc.vector.tensor_copy(out=st[:], in_=pt[:])
                row0 = (sc * (SC // 128) + i) * 128
                nc.sync.dma_start(out=y1t_d[row0:row0 + 128, :], in_=st[:])

        # ---- per word-chunk: transpose-mode gather of ntaps rows + max tree
        # out[p, c, i] = y1t[idx[i]][c*128+p]; idx[j*128+wl] = clamp(s+j, e)
        a2_all = const.tile([128, KCH, W], BF16, name="a2_all")
        for wc in range(8 if stage >= 7 else 0):
            tap = gat.tile([128, KCH, ntaps * 128], BF16, tag="tap", name="tap")
            # words wc*128..(wc+1)*128-1 end at byte <= ntaps*(w+1)-1
            rmax = min(((ntaps * 128 * (wc + 1) + 127) // 128) * 128, T)
            nc.gpsimd.dma_gather(
                out_ap=tap[:],
                in_ap=y1t_d[0:rmax, :],
                idxs_ap=gidx_sb[:, wc * ntaps * 8:(wc + 1) * ntaps * 8],
                num_idxs=ntaps * 128,
                num_idxs_reg=ntaps * 128,
                elem_size=DW,
                transpose=True,
                single_packet=False,
            )
            a2s = a2_all[:, :, wc * 128:(wc + 1) * 128]
            nc.vector.tensor_tensor(
                out=a2s, in0=tap[:, :, 0:128], in1=tap[:, :, 128:256], op=OP.max)
            for j in range(2, ntaps):
                nc.vector.tensor_tensor(
                    out=a2s, in0=a2s, in1=tap[:, :, j * 128:(j + 1) * 128],
                    op=OP.max)

        if stage < 8:
            nc.sync.dma_start(out=dbg_d[4], in_=xg[:])
            for m in range(MCH):
                nc.sync.dma_start(out=dbg_d[m], in_=y0_sb[m][:])

        # ---- proj + bias (ones-row matmul), fp32 out ----
        for wc in range(8 if stage >= 8 else 0):
            po = ps.tile([128, DW], F32, tag="ps", name="po")
            for k in range(KCH):
                nc.tensor.matmul(
                    out=po[:],
                    lhsT=a2_all[:, k, wc * 128:(wc + 1) * 128],
                    rhs=projw_sb[:, k, :],
                    start=(k == 0),
                    stop=False,
                )
            nc.tensor.matmul(
                out=po[:], lhsT=ones_sb[:, 0:128], rhs=projb_sb[:],
                start=False, stop=True,
            )
            ob = obp.tile([128, DW], F32, tag="ob", name="ob")
            nc.vector.tensor_copy(out=ob[:], in_=po[:])
            nc.sync.dma_start(out=out_d[wc * 128:(wc + 1) * 128, :], in_=ob[:])

    nc.compile()
    return nc


@functools.lru_cache(maxsize=2)
def _program(ntaps: int) -> bass.Bass:
    return build_program(ntaps)


def _pack_idx(lin: np.ndarray) -> np.ndarray:
    """SWDGE idx layout: [128, N/16] int16, value n at [p, s] with
    n = s*16 + p%16, replicated across the eight 16-partition groups."""
    n = len(lin)
    assert n % 16 == 0
    arr = np.asarray(lin, dtype=np.int16).reshape(n // 16, 16).T  # [16, n/16]
    return np.tile(arr, (8, 1)).copy()


def prepare(inputs):
    f32 = np.float32
    bt = np.asarray(inputs["byte_tokens"]).astype(np.int64)
    bpe = np.asarray(inputs["bpe_mask"]).astype(np.int64)
    wrd = np.asarray(inputs["word_mask"]).astype(np.int64)
    seg = np.asarray(inputs["seg_ids"]).astype(np.int64)
    emb = np.asarray(inputs["tok_emb"], dtype=f32)
    conv0_w = np.asarray(inputs["conv0_w"], dtype=f32)
    conv0_b = np.asarray(inputs["conv0_b"], dtype=f32)
    conv1_w = np.asarray(inputs["conv1_w"], dtype=f32)
    conv1_b = np.asarray(inputs["conv1_b"], dtype=f32)
    hw_w = {
        (0, "g"): np.asarray(inputs["hw0_wg"], dtype=f32),
        (0, "h"): np.asarray(inputs["hw0_wh"], dtype=f32),
        (1, "g"): np.asarray(inputs["hw1_wg"], dtype=f32),
        (1, "h"): np.asarray(inputs["hw1_wh"], dtype=f32),
    }
    hw_b = {
        (0, "g"): np.asarray(inputs["hw0_bg"], dtype=f32),
        (0, "h"): np.asarray(inputs["hw0_bh"], dtype=f32),
        (1, "g"): np.asarray(inputs["hw1_bg"], dtype=f32),
        (1, "h"): np.asarray(inputs["hw1_bh"], dtype=f32),
    }
    proj_w = np.asarray(inputs["proj_w"], dtype=f32)
    proj_b = np.asarray(inputs["proj_b"], dtype=f32)

    def as_bf16(x):
        return np.ascontiguousarray(x.astype(bf16_np))

    # combined embedding table: row v + 264*(b + 2*w) = E[v] + b*E4 + w*E3
    embc = np.zeros((CVOCAB, 128), f32)
    for bm in range(2):
        for wm in range(2):
            r0 = VOCAB * (bm + 2 * wm)
            embc[r0:r0 + VOCAB, :DB] = (
                emb + bm * emb[BPE_MARK] + wm * emb[WORD_MARK])
    c1 = conv1_w.copy()
    c1[1] += np.eye(DW, dtype=f32)

    def chunk_kl(wm):  # [L, 512, 512] -> [128, L, 4, 512]
        L = wm.shape[0]
        return np.transpose(wm.reshape(L, KCH, 128, DW), (2, 0, 1, 3))

    shared = {
        "emb_comb": as_bf16(embc),
        "w0": as_bf16(np.transpose(conv0_w, (1, 0, 2))),          # [64,3,512]
        "w1": as_bf16(np.transpose(c1.reshape(3, KCH, 128, DW), (2, 0, 1, 3))),
        "wg0": as_bf16(chunk_kl(hw_w[(0, "g")])),
        "wh0": as_bf16(chunk_kl(hw_w[(0, "h")])),
        "wg1": as_bf16(chunk_kl(hw_w[(1, "g")])),
        "wh1": as_bf16(chunk_kl(hw_w[(1, "h")])),
        "projw": as_bf16(np.transpose(proj_w.reshape(KCH, 128, DW), (1, 0, 2))),
        "projb": as_bf16(proj_b.reshape(1, DW)),
        "ident": np.eye(128, dtype=bf16_np),
    }

    bias_h = np.zeros((128, 40), f32)
    bias_h[:, 0:4] = conv0_b.reshape(KCH, 128).T
    bias_h[:, 4:8] = conv1_b.reshape(KCH, 128).T
    for blk in (0, 1):
        for l in range(NH):
            bias_h[:, _col_bg(blk, l):_col_bg(blk, l) + 4] = \
                hw_b[(blk, "g")][l].reshape(KCH, 128).T
            bias_h[:, _col_bh(blk, l):_col_bh(blk, l) + 4] = \
                hw_b[(blk, "h")][l].reshape(KCH, 128).T
    shared["biases"] = np.ascontiguousarray(bias_h)

    # per-core seg prep; ntaps = max segment length over the whole batch
    counts = np.zeros((B, W), np.int64)
    for b in range(B):
        counts[b] = np.bincount(seg[b], minlength=W)[:W]
    assert (counts >= 1).all(), "empty segments unsupported"
    ntaps = max(int(counts.max()), 2)
    starts = np.zeros((B, W), np.int64)
    starts[:, 1:] = np.cumsum(counts, axis=1)[:, :-1]
    ends = starts + counts - 1

    in_maps = []
    for b in range(B):
        cidx = bt[b] + VOCAB * (bpe[b] + 2 * wrd[b])
        gl = np.empty(8 * ntaps * 128, np.int64)
        for wc in range(8):
            nvec = np.arange(ntaps * 128)
            wv = wc * 128 + (nvec % 128)
            jv = nvec // 128
            gl[wc * ntaps * 128:(wc + 1) * ntaps * 128] = np.minimum(
                starts[b, wv] + jv, ends[b, wv]
            )
        m = dict(shared)
        m["tok_idx"] = _pack_idx(cidx)
        m["gidx"] = np.concatenate(
            [_pack_idx(gl[wc * ntaps * 128:(wc + 1) * ntaps * 128])
             for wc in range(8)], axis=1
        ).copy()
        in_maps.append(m)
    return ntaps, in_maps


def _run(inputs, trace=False, **kwargs):
    ntaps, in_maps = prepare(inputs)
    nc = _program(ntaps)
    res = run_bass_kernel_spmd(
        nc, in_maps, core_ids=list(range(NCORES)), trace=trace, **kwargs
    )
    out = np.stack([res.results[b]["out"] for b in range(B)], axis=0)
    return out.astype(np.float32), res


def kernel(**inputs) -> np.ndarray:
    out, _ = _run(inputs, trace=False)
    return out


def run_traced(inputs, **kwargs):
    return _run(inputs, trace=True, **kwargs)
